# revision 5
# baseline (speedup 1.0000x reference)
"""AdaptiveSoftmax log-prob kernel for 8 TRN2 NeuronCores (Bass/Tile), v4.

v4 = v3 (host-side token routing, vocab-parallel fp8 DoubleRow GEMMs)
restructured into two phases:

Phase H (all 16 token tiles, needs only x + head_w which load first):
  head mm -> headstaged drain (DVE) -> exp (ACT, from PSUM, accum s_h).
  Runs concurrently with the 5 MB t0_b weight stream.  One AllReduce at
  the end carries all 16 tiles' (s_h, l9999*own, l10000*own).

Phase T (NT0 tail0 tiles in blocks of 2): psB/C/D mms -> drains (DVE;
  psD on ACT) -> t0 exp (ACT, wide, from staged) -> per-block AllReduce
  of (s_t0, s_t1) only -> lag-1 stats (Pool + one ACT Ln) -> epilogue
  (DVE + Pool spans; ACT joins for the tail blocks) -> output DMA.
  Head epilogues (add -ln s_h) interleave, 2-3 per block, on DVE.

The out[10000:10002] boundary cols for non-tail0 rows are filled on the
host from core 7's returned head slice.
"""

import numpy as np
import ml_dtypes

import concourse.bass as bass
import concourse.mybir as mybir
import concourse.tile as tile
from concourse import bacc
from concourse.bass_utils import run_bass_kernel_spmd

BF16 = mybir.dt.bfloat16
F32 = mybir.dt.float32
F16 = mybir.dt.float16
FP8 = mybir.dt.float8e4
AF = mybir.ActivationFunctionType
ALU = mybir.AluOpType
DR = mybir.MatmulPerfMode.DoubleRow

P = 128
D = 1024
KT = D // P
NKP = KT // 2
NTOK = 2048
NCORES = 8
NTILE = NTOK // P    # 16
TPB = 2

VOCAB = 50257
CUT0, CUT1 = 10000, 50000
HEAD_REAL = 10002
HEAD_W = 1252
HEAD_PAD = HEAD_W * NCORES - HEAD_REAL       # 14
T0_W = 5000
T1_REAL = VOCAB - CUT1
T1_W = 33
T1_PAD = T1_W * NCORES - T1_REAL             # 7
T1OFF = HEAD_W + T0_W
OUT_W = HEAD_W + T0_W + T1_W                 # 6285
TS_W = T0_W + T1_W                           # t0staged width 5033

SX = 8.0
SW = 32.0
SL = SX * SW
SPROJ = SX / SL

E_OFF = 9999 - 7 * HEAD_W    # 1235: cols l9999,l10000 on core 7

DEF_NT0 = 13
DEF_T1LO = 12
DEF_T1HI = 13

_BUILT = None          # latest built nc (test.py chain_slope reads this)
_BUILT_KEY = None


def _routing(tgt):
    m0 = (tgt >= CUT0) & (tgt < CUT1)
    m1 = tgt >= CUT1
    n0 = int(m0.sum())
    n1 = int(m1.sum())
    perm = np.concatenate([np.nonzero(m0)[0], np.nonzero(m1)[0],
                           np.nonzero(~(m0 | m1))[0]])
    nt0 = max(1, -(-n0 // P))
    t1lo = n0 // P
    t1hi = max(t1lo + 1, -(-(n0 + n1) // P)) if n1 else t1lo
    return perm, n0, n1, nt0, t1lo, t1hi


def _fix_act_tables(nc):
    from concourse.hw_specs import get_activation_tables
    tabs = get_activation_tables(nc.m.arch)
    need = {AF.Exp, AF.Ln, AF.Copy, AF.Identity}
    target = None
    for i, (name, funcs) in enumerate(tabs.items()):
        if need.issubset(funcs):
            target = i
            break
    if target is None:
        return
    first = True
    for blk in nc.m.functions[0].blocks:
        drop = []
        for idx, inst in enumerate(blk.instructions):
            if not isinstance(inst, mybir.InstLoadActFuncSet):
                continue
            si = inst.sync_info
            if si is not None and (len(si.on_wait) or len(si.on_update)):
                continue
            if first:
                inst.act_func_set_id = target
                first = False
            else:
                drop.append(idx)
        for idx in reversed(drop):
            del blk.instructions[idx]


def build_nc(use_collective=True, dedup_proj=True,
             nt0=DEF_NT0, t1lo=DEF_T1LO, t1hi=DEF_T1HI, n0=1651,
             epi_dve=1100, epi_act=1100, tail_dve=2100, tail_act=1300,
             lag=2, special_last=False, h_act_drains=8, x0_first=False,
             psd_dve=0, tail_nblk=2, startup_fine=False,
             last_singles=False):
    nc = bacc.Bacc("TRN2", target_bir_lowering=False, debug=False,
                   num_devices=NCORES)

    nt0 = max(nt0, t1hi)          # t1-only tiles ride the t0 pipeline
    special = [t for t in range(nt0) if (t + 1) * P > n0 or
               (t1lo <= t < t1hi)]
    plain = [t for t in range(nt0) if t not in special]
    if last_singles and len(plain) >= 2:
        pairs = [plain[i:i + TPB] for i in range(0, len(plain) - 2, TPB)]
        pairs += [[plain[-2]], [plain[-1]]]
    else:
        pairs = [plain[i:i + TPB] for i in range(0, len(plain), TPB)]
    if special_last:
        blocks = pairs + [special] if special else pairs
    else:
        blocks = ([special] if special else []) + pairs
    nblk = len(blocks)
    head_only = list(range(nt0, NTILE))

    def has_t1(t):
        return t1lo <= t < t1hi

    xT = nc.dram_tensor("xT", [D, NTOK], FP8, kind="ExternalInput")
    hwT = nc.dram_tensor("hwT", [D, HEAD_W], FP8, kind="ExternalInput")
    t0bT = nc.dram_tensor("t0bT", [D, T0_W], FP8, kind="ExternalInput")
    t1bT = nc.dram_tensor("t1bT", [D // 4, T1_W], FP8, kind="ExternalInput")
    t0aT = nc.dram_tensor("t0aT", [D, P], FP8, kind="ExternalInput")
    t1aT = nc.dram_tensor("t1aT", [D, D // 4], FP8, kind="ExternalInput")
    masks = nc.dram_tensor("masks", [P, NTILE, 2], F32, kind="ExternalInput")
    flags = nc.dram_tensor("flags", [P, 2], F32, kind="ExternalInput")
    out = nc.dram_tensor("out", [NTOK, OUT_W], F16, kind="ExternalOutput")

    xT_r = xT.rearrange("(k p) t -> p k t", p=P)
    hwT_r = hwT.rearrange("(k p) n -> p k n", p=P)
    t0bT_r = t0bT.rearrange("(k p) n -> p k n", p=P)
    t1bT_r = t1bT.rearrange("(k p) n -> p k n", p=P)
    t0aT_r = t0aT.rearrange("(k p) n -> p k n", p=P)
    t1aT_r = t1aT.rearrange("(k p) n -> p k n", p=P)

    rg = [list(range(NCORES))]
    nb = TPB * P

    with tile.TileContext(nc) as tc:
        with (
            tc.tile_pool(name="const", bufs=1) as const,
            tc.tile_pool(name="projpool", bufs=2) as projpool,
            tc.tile_pool(name="stpool", bufs=lag + 2) as stpool,
            tc.tile_pool(name="deadpool", bufs=1) as deadpool,
            tc.tile_pool(name="accpool", bufs=lag + 2) as accpool,
            tc.tile_pool(name="scpool", bufs=2) as scpool,
            tc.tile_pool(name="psum", bufs=4, space="PSUM") as psum,
            tc.tile_pool(name="dram", bufs=3, space="DRAM") as dram,
        ):
            # ---- resident inputs; order = phase-H needs first ----
            xT_sb = const.tile([P, KT, NTOK], FP8)
            hw_sb = const.tile([P, KT, HEAD_W], FP8)
            if startup_fine:
                nc.sync.dma_start(hw_sb[:, :, 0:512], hwT_r[:, :, 0:512])
                nc.sync.dma_start(xT_sb[:, :, 0:128], xT_r[:, :, 0:128])
                nc.sync.dma_start(hw_sb[:, :, 512:1024],
                                  hwT_r[:, :, 512:1024])
                nc.sync.dma_start(hw_sb[:, :, 1024:HEAD_W],
                                  hwT_r[:, :, 1024:HEAD_W])
                for c0 in range(128, NTOK, 512):
                    c1 = min(c0 + 512, NTOK)
                    nc.sync.dma_start(xT_sb[:, :, c0:c1], xT_r[:, :, c0:c1])
            else:
                if x0_first:
                    nc.sync.dma_start(xT_sb[:, :, 0:256], xT_r[:, :, 0:256])
                nc.sync.dma_start(hw_sb[:], hwT_r)
                for c0 in range(256 if x0_first else 0, NTOK, 512):
                    c1 = min(c0 + 512, NTOK)
                    nc.sync.dma_start(xT_sb[:, :, c0:c1],
                                      xT_r[:, :, c0:c1])
            t0a_sb = const.tile([P, KT, P], FP8)
            nc.sync.dma_start(t0a_sb[:], t0aT_r)
            t1a_sb = const.tile([P, KT, D // 4], FP8)
            nc.sync.dma_start(t1a_sb[:], t1aT_r)
            t1b_sb = const.tile([P, 2, T1_W], FP8)
            nc.sync.dma_start(t1b_sb[:], t1bT_r)
            masks_sb = const.tile([P, NTILE, 2], F32)
            nc.sync.dma_start(masks_sb[:], masks[:])
            flags_sb = const.tile([P, 2], F32)
            nc.sync.dma_start(flags_sb[:], flags[:])
            t0b_sb = const.tile([P, KT, T0_W], FP8)
            for c0 in range(0, T0_W, 1024):
                c1 = min(c0 + 1024, T0_W)
                nc.sync.dma_start(t0b_sb[:, :, c0:c1], t0bT_r[:, :, c0:c1])
            zero_b = const.tile([P, 1], F32)
            nc.vector.memset(zero_b[:], 0.0)

            headstaged = const.tile([P, NTILE, HEAD_W], F16)
            arH = const.tile([P, NTILE, 3], F32)
            chH = const.tile([P, NTILE], F32)
            lnH = const.tile([P, NTILE], F32)

            def mm_chunk(ps, pslo, lhs, w_sb, woff, width, nkp=NKP):
                for s0 in range(0, width, 512):
                    w = min(512, width - s0)
                    for j in range(nkp):
                        nc.tensor.matmul(
                            ps[:, pslo + s0:pslo + s0 + w],
                            lhs[:, 2 * j:2 * j + 2, :],
                            w_sb[:, 2 * j:2 * j + 2, woff + s0:woff + s0 + w],
                            start=(j == 0), stop=(j == nkp - 1), perf_mode=DR)

            # ================= phase H =================
            def emit_h(t):
                xt = xT_sb[:, :, t * P:(t + 1) * P]
                psA = psum.tile([P, 1024], F32, tag="mm")
                mm_chunk(psA, 0, xt, hw_sb, 0, 1024)
                nc.vector.tensor_scalar_mul(headstaged[:, t, 0:1024],
                                            psA[:], 1.0 / SL)
                psA2 = psum.tile([P, 1024], F32, tag="mm")
                mm_chunk(psA2, 0, xt, hw_sb, 1024, HEAD_W - 1024)
                h_set = ({int(i * NTILE / h_act_drains)
                          for i in range(h_act_drains)}
                         if h_act_drains else set())
                if t in h_set:
                    nc.scalar.activation(headstaged[:, t, 1024:HEAD_W],
                                         psA2[:, 0:HEAD_W - 1024], AF.Copy,
                                         scale=1.0 / SL)
                else:
                    nc.vector.tensor_scalar_mul(
                        headstaged[:, t, 1024:HEAD_W],
                        psA2[:, 0:HEAD_W - 1024], 1.0 / SL)
                dead = deadpool.tile([P, T0_W], F16, tag="dead")
                nc.scalar.activation(dead[:, 0:HEAD_W], headstaged[:, t, :],
                                     AF.Exp, bias=zero_b[:],
                                     accum_out=arH[:, t, 0:1])

            for t in range(10):
                emit_h(t)

            # proj1 for t1 tiles (needs only x + t1a)
            proj1T = None
            nt1 = t1hi - t1lo
            if nt1 > 0:
                proj1T = const.tile([P, 2, nt1 * P], FP8)
                ps1 = psum.tile([P, 1024], F32, tag="mm")
                for kd in range(2):
                    lhs = t1a_sb[:, :, kd * P:(kd + 1) * P]
                    for i in range(nt1):
                        tlo = (t1lo + i) * P
                        for j in range(NKP):
                            nc.tensor.matmul(
                                ps1[:, kd * nt1 * P + i * P:
                                    kd * nt1 * P + (i + 1) * P],
                                lhs[:, 2 * j:2 * j + 2, :],
                                xT_sb[:, 2 * j:2 * j + 2, tlo:tlo + P],
                                start=(j == 0), stop=(j == NKP - 1),
                                perf_mode=DR)
                for kd in range(2):
                    nc.vector.tensor_scalar_mul(
                        proj1T[:, kd, :],
                        ps1[:, kd * nt1 * P:(kd + 1) * nt1 * P], SPROJ)

            early_proj0 = [None]

            def emit_rest_of_h():
                for t in range(10, NTILE):
                    emit_h(t)

            # gated head logits l9999,l10000 for every tile, then head-AR
            arrH = const.tile([P, NTILE, 3], F32)

            def finish_h():
                nc.gpsimd.tensor_scalar_mul(
                    arH[:, :, 1:3], headstaged[:, :, E_OFF:E_OFF + 2],
                    flags_sb[:, 0:1])
                arH_in = dram.tile([P, NTILE, 3], F32, tag="arH_in")
                arH_out = dram.tile([P, NTILE, 3], F32, tag="arH_out",
                                    addr_space="Shared")
                nc.sync.dma_start(arH_in[:], arH[:])
                if use_collective:
                    nc.gpsimd.collective_compute(
                        "AllReduce", ALU.add, replica_groups=rg,
                        ins=[arH_in[:]], outs=[arH_out[:]])
                else:
                    nc.sync.dma_start(arH_out[:], arH_in[:])
                nc.sync.dma_start(arrH[:], arH_out[:])

            # ================= phase T =================
            def emit_proj(b):
                w = len(blocks[b]) * P
                projT = projpool.tile([P, KT, nb], FP8, tag="projT")
                ps = psum.tile([P, 1024], F32, tag="mm")
                for i, t in enumerate(blocks[b]):
                    tlo = t * P
                    for j in range(NKP):
                        nc.tensor.matmul(
                            ps[:, i * P:(i + 1) * P],
                            t0a_sb[:, 2 * j:2 * j + 2, :],
                            xT_sb[:, 2 * j:2 * j + 2, tlo:tlo + P],
                            start=(j == 0), stop=(j == NKP - 1), perf_mode=DR)
                pslice = projpool.tile([P, nb], FP8, tag="pslice")
                nc.vector.tensor_scalar_mul(pslice[:], ps[:, 0:nb], SPROJ)
                ag_in = dram.tile([P, nb], FP8, tag="ag_in")
                ag_out = dram.tile([D, nb], FP8, tag="ag_out",
                                   addr_space="Shared")
                nc.sync.dma_start(ag_in[:], pslice[:])
                if use_collective:
                    nc.gpsimd.collective_compute(
                        "AllGather", ALU.bypass, replica_groups=rg,
                        ins=[ag_in[:]], outs=[ag_out[:]])
                else:
                    nc.sync.dma_start(
                        ag_out.rearrange("(kd p) t -> p kd t", p=P)[:, 0],
                        ag_in[:])
                nc.sync.dma_start(
                    projT[:],
                    ag_out.rearrange("(kd p) t -> p kd t", p=P))
                return projT

            def emit_tile(b, i, projT, staged, ar):
                t = blocks[b][i]
                pj = projT[:, :, i * P:(i + 1) * P]
                dead = deadpool.tile([P, T0_W], F16, tag="dead")
                for q in range(4):
                    psq = psum.tile([P, 1024], F32, tag="mm")
                    mm_chunk(psq, 0, pj, t0b_sb, q * 1024, 1024)
                    nc.vector.tensor_scalar_mul(
                        staged[:, i, q * 1024:(q + 1) * 1024], psq[:],
                        1.0 / SL)
                psD = psum.tile([P, 1024], F32, tag="mm")
                mm_chunk(psD, 0, pj, t0b_sb, 4096, T0_W - 4096)
                if has_t1(t):
                    pj1 = proj1T[:, :, (t - t1lo) * P:(t - t1lo + 1) * P]
                    mm_chunk(psD, T0_W - 4096, pj1, t1b_sb, 0, T1_W, nkp=1)
                    nc.vector.tensor_scalar_mul(
                        staged[:, i, T0_W:TS_W],
                        psD[:, T0_W - 4096:T0_W - 4096 + T1_W], 1.0 / SL)
                    t1dead = deadpool.tile([P, T1_W], F16, tag="t1dead")
                    nc.scalar.activation(t1dead[:],
                                         psD[:, T0_W - 4096:
                                             T0_W - 4096 + T1_W],
                                         AF.Exp, bias=zero_b[:],
                                         scale=1.0 / SL,
                                         accum_out=ar[:, i, 1:2])
                # psD drain (ACT by default; DVE on alternate tiles)
                if psd_dve and t % psd_dve == 0:
                    nc.vector.tensor_scalar_mul(staged[:, i, 4096:T0_W],
                                                psD[:, 0:T0_W - 4096],
                                                1.0 / SL)
                else:
                    nc.scalar.activation(staged[:, i, 4096:T0_W],
                                         psD[:, 0:T0_W - 4096], AF.Copy,
                                         scale=1.0 / SL)
                nc.scalar.activation(dead[:, 0:T0_W], staged[:, i, 0:T0_W],
                                     AF.Exp, bias=zero_b[:],
                                     accum_out=ar[:, i, 0:1])

            def finish_ar(b, ar):
                ar_in = dram.tile([P, TPB, 2], F32, tag="ar_in")
                ar_out = dram.tile([P, TPB, 2], F32, tag="ar_out",
                                   addr_space="Shared")
                nc.sync.dma_start(ar_in[:], ar[:])
                if use_collective:
                    nc.gpsimd.collective_compute(
                        "AllReduce", ALU.add, replica_groups=rg,
                        ins=[ar_in[:]], outs=[ar_out[:]])
                else:
                    nc.sync.dma_start(ar_out[:], ar_in[:])
                arr = accpool.tile([P, TPB, 2], F32, tag="arr")
                nc.sync.dma_start(arr[:], ar_out[:])
                return arr

            def head_stats():
                """chH[t] = -ln(s_h[t] - HEAD_PAD) for all 16 tiles."""
                nc.gpsimd.tensor_scalar_add(lnH[:], arrH[:, :, 0],
                                            -float(HEAD_PAD))
                nc.scalar.activation(lnH[:], lnH[:], AF.Ln, bias=zero_b[:])
                nc.gpsimd.tensor_scalar_mul(chH[:], lnH[:], -1.0)

            def head_epi(t):
                nc.vector.tensor_scalar_add(headstaged[:, t, :],
                                            headstaged[:, t, :],
                                            chH[:, t:t + 1])
                nc.sync.dma_start(out[t * P:(t + 1) * P, 0:HEAD_W],
                                  headstaged[:, t, :])

            def epi_stats(b, arr, sc):
                """c0 = l9999 + chH - ln s_t0 ; c1 = l10000 + chH -
                ln(s_t1 - T1_PAD).  Pool + one ACT Ln."""
                lns_in, lns = sc["lns_in"], sc["lns"]
                c0, c1, u = sc["c0"], sc["c1"], sc["u"]
                ts = blocks[b][0]
                te = ts + len(blocks[b])
                nc.gpsimd.tensor_copy(lns_in[:, :len(blocks[b]), 0],
                                      arr[:, :len(blocks[b]), 0])
                nc.gpsimd.tensor_scalar_add(lns_in[:, :, 1], arr[:, :, 1],
                                            -float(T1_PAD))
                nc.scalar.activation(lns[:], lns_in[:], AF.Ln,
                                     bias=zero_b[:])
                nc.gpsimd.tensor_add(u[:, :len(blocks[b])],
                                     arrH[:, ts:te, 1], chH[:, ts:te])
                nc.gpsimd.tensor_sub(c0[:], u[:], lns[:, :, 0])
                nc.gpsimd.tensor_add(u[:, :len(blocks[b])],
                                     arrH[:, ts:te, 2], chH[:, ts:te])
                nc.gpsimd.tensor_sub(c1[:], u[:], lns[:, :, 1])

            def epi_apply(b, staged, sc, dve_w, act_w):
                c0, c1 = sc["c0"], sc["c1"]
                pool_lo = dve_w + act_w
                for i, t in enumerate(blocks[b]):
                    tlo = t * P
                    masked = (t + 1) * P > n0
                    m0 = masks_sb[:, t, 0:1]
                    if masked:
                        nc.vector.tensor_scalar(
                            staged[:, i, 0:dve_w], staged[:, i, 0:dve_w],
                            c0[:, i:i + 1], m0, ALU.add, ALU.mult)
                        nc.gpsimd.tensor_scalar(
                            staged[:, i, pool_lo:T0_W],
                            staged[:, i, pool_lo:T0_W],
                            c0[:, i:i + 1], m0, ALU.add, ALU.mult)
                        if act_w:
                            # ACT has no fused mask: bias-add then Pool mask
                            nc.scalar.activation(
                                staged[:, i, dve_w:pool_lo],
                                staged[:, i, dve_w:pool_lo], AF.Identity,
                                bias=c0[:, i:i + 1])
                            nc.gpsimd.tensor_scalar_mul(
                                staged[:, i, dve_w:pool_lo],
                                staged[:, i, dve_w:pool_lo], m0)
                    else:
                        nc.vector.tensor_scalar_add(
                            staged[:, i, 0:dve_w], staged[:, i, 0:dve_w],
                            c0[:, i:i + 1])
                        if act_w:
                            nc.scalar.activation(
                                staged[:, i, dve_w:pool_lo],
                                staged[:, i, dve_w:pool_lo], AF.Identity,
                                bias=c0[:, i:i + 1])
                        nc.gpsimd.tensor_scalar_add(
                            staged[:, i, pool_lo:T0_W],
                            staged[:, i, pool_lo:T0_W], c0[:, i:i + 1])
                    width = T0_W
                    if has_t1(t):
                        width = TS_W
                        m1 = masks_sb[:, t, 1:2]
                        nc.vector.tensor_scalar(
                            staged[:, i, T0_W:TS_W],
                            staged[:, i, T0_W:TS_W],
                            c1[:, i:i + 1], m1, ALU.add, ALU.mult)
                    nc.sync.dma_start(out[tlo:tlo + P, HEAD_W:HEAD_W + dve_w],
                                      staged[:, i, 0:dve_w])
                    nc.sync.dma_start(
                        out[tlo:tlo + P, HEAD_W + dve_w:HEAD_W + width],
                        staged[:, i, dve_w:width])

            def new_sc():
                shapes = {"lns_in": [P, TPB, 2], "lns": [P, TPB, 2],
                          "c0": [P, TPB], "c1": [P, TPB], "u": [P, TPB]}
                return {k: scpool.tile(s, F32, tag=k, name=k)
                        for k, s in shapes.items()}

            state = {}
            projT = emit_proj(0)
            emit_rest_of_h()
            finish_h()
            for bi in range(nblk + lag):
                nxt = emit_proj(bi + 1) if bi + 1 < nblk else None
                if bi < nblk:
                    staged = stpool.tile([P, TPB, TS_W], F16, tag="staged")
                    ar = accpool.tile([P, TPB, 2], F32, tag="ar")
                    state[bi] = {"staged": staged, "ar": ar}
                    for i in range(len(blocks[bi])):
                        emit_tile(bi, i, projT, staged, ar)
                    projT = nxt
                if bi < nblk:
                    state[bi]["arr"] = finish_ar(bi, ar)
                if bi == 0:
                    head_stats()
                    for t in head_only:
                        head_epi(t)
                if bi - lag in state:
                    b = bi - lag
                    s = state.pop(b)
                    sc = new_sc()
                    epi_stats(b, s["arr"], sc)
                    tail = b >= nblk - tail_nblk
                    epi_apply(b, s["staged"], sc,
                              tail_dve if tail else epi_dve,
                              tail_act if tail else epi_act)
                    for t in blocks[b]:
                        head_epi(t)

    nc.compile()
    _fix_act_tables(nc)
    return nc


def _fp8(a, scale):
    return np.ascontiguousarray(a * scale).astype(ml_dtypes.float8_e4m3)


def make_in_maps(input, target, head_w, t0_a, t0_b, t1_a, t1_b,
                 dedup_proj=True):
    x = np.asarray(input, dtype=np.float32).reshape(NTOK, D)
    tgt = np.asarray(target).reshape(NTOK)
    head_w = np.asarray(head_w, dtype=np.float32)
    t0_a = np.asarray(t0_a, dtype=np.float32)
    t0_b = np.asarray(t0_b, dtype=np.float32)
    t1_a = np.asarray(t1_a, dtype=np.float32)
    t1_b = np.asarray(t1_b, dtype=np.float32)

    perm, n0, n1, nt0, t1lo, t1hi = _routing(tgt)
    xp = x[perm]
    tp = tgt[perm]

    xT = _fp8(xp.T, SX)
    hw_pad = np.zeros((HEAD_W * NCORES, D), np.float32)
    hw_pad[:HEAD_REAL] = head_w
    hwT = _fp8(hw_pad.T, SW)
    t0bT = _fp8(t0_b.T, SW)
    t1_pad = np.zeros((T1_W * NCORES, D // 4), np.float32)
    t1_pad[:T1_REAL] = t1_b
    t1bT = _fp8(t1_pad.T, SW)
    t0aT = _fp8(t0_a.T, SW)
    t1aT = _fp8(t1_a.T, SW)

    m0 = ((tp >= CUT0) & (tp < CUT1)).astype(np.float32)
    m1 = (tp >= CUT1).astype(np.float32)
    masks = np.stack([m0.reshape(NTILE, P).T,
                      m1.reshape(NTILE, P).T], axis=-1)
    masks = np.ascontiguousarray(masks, dtype=np.float32)

    in_maps = []
    for c in range(NCORES):
        flags = np.zeros((P, 2), np.float32)
        flags[:, 0] = 1.0 if c == NCORES - 1 else 0.0
        in_maps.append({
            "xT": xT,
            "hwT": np.ascontiguousarray(hwT[:, c * HEAD_W:(c + 1) * HEAD_W]),
            "t0bT": np.ascontiguousarray(t0bT[:, c * T0_W:(c + 1) * T0_W]),
            "t1bT": np.ascontiguousarray(t1bT[:, c * T1_W:(c + 1) * T1_W]),
            "t0aT": np.ascontiguousarray(t0aT[:, c * P:(c + 1) * P]),
            "t1aT": t1aT,
            "masks": masks,
            "flags": flags,
        })
    return in_maps


def assemble(results, perm, n0, n1, nt0, t1lo, t1hi):
    full = np.zeros((NTOK, VOCAB), np.float32)
    outs = [np.asarray(results[c]["out"], dtype=np.float32)
            for c in range(NCORES)]
    for c in range(NCORES):
        lo = c * HEAD_W
        hi = min((c + 1) * HEAD_W, CUT0)
        if hi > lo:
            full[perm, lo:hi] = outs[c][:, :hi - lo]
    r0 = perm[:nt0 * P]
    for c in range(NCORES):
        full[r0[:, None], CUT0 + c * T0_W + np.arange(T0_W)[None, :]] = \
            outs[c][:nt0 * P, HEAD_W:HEAD_W + T0_W]
    c7 = NCORES - 1
    off = CUT0 - c7 * HEAD_W
    nont0 = perm[n0:]
    full[nont0[:, None], CUT0 + np.arange(2)[None, :]] = \
        outs[c7][n0:, off:off + 2]
    if t1hi > t1lo:
        r1 = perm[t1lo * P:t1hi * P]
        rows = slice(t1lo * P, t1hi * P)
        for c in range(NCORES):
            lo1 = CUT1 + c * T1_W
            hi1 = min(lo1 + T1_W, VOCAB)
            if hi1 > lo1:
                full[r1[:, None], lo1 + np.arange(hi1 - lo1)[None, :]] = \
                    outs[c][rows, T1OFF:T1OFF + hi1 - lo1]
    return full.reshape(2, NTOK // 2, VOCAB)


def kernel(input, target, head_w, t0_a, t0_b, t1_a, t1_b):
    global _BUILT, _BUILT_KEY
    tgt = np.asarray(target).reshape(NTOK)
    perm, n0, n1, nt0, t1lo, t1hi = _routing(tgt)
    key = (True, nt0, t1lo, t1hi, n0)
    if _BUILT_KEY != key:
        _BUILT = build_nc(use_collective=True,
                          nt0=nt0, t1lo=t1lo, t1hi=t1hi, n0=n0)
        _BUILT_KEY = key
    nc = _BUILT
    in_maps = make_in_maps(input, target, head_w, t0_a, t0_b, t1_a, t1_b)
    res = run_bass_kernel_spmd(nc, in_maps, core_ids=list(range(NCORES)))
    return assemble(res.results, perm, n0, n1, nt0, t1lo, t1hi)


if __name__ == "__main__":
    import time
    t0 = time.time()
    nc = build_nc(use_collective=False)
    print(f"build+compile: {time.time() - t0:.1f}s")
